# revision 7
# baseline (speedup 1.0000x reference)
"""Trainium2 Bass kernel for nn_Bsl2_9053791060551 (bi-GRU + segment reduce + MLP).

Self-contained: builds a Bass/Tile program per call and runs it SPMD on 8
NeuronCores, data-parallel over batch (8 sequences per core).

v2 design notes (per core, feature-major):
  - tokens tau = t*8 + b  (t-major interleave of the 8 local sequences)
  - gate biases and the input projections are written DIRECTLY into the
    scan's PSUM banks via matmuls (bias via K=4/K=2 indicator matmuls,
    projection accumulated with start=False).  The recurrent matmuls then
    accumulate on top, so the scan's elementwise chain is just:
      sig = sigmoid(psum_rz); t1 = psum_n*r; t2 = t1+xi_n; n = tanh(t2)
      dt = prev-n; et = z*dt; h = n+et
    (7 ops per direction-step vs 16 in v1).
  - fwd/bwd chains stay separate so their elementwise latencies hide
    behind each other's recurrent matmuls.
  - hT flush + DMA-transpose to token-major h_tok happen per iteration,
    overlapped with the scan (v1 had a separate 220us transpose phase).
  - gathers batched 4x ([128,256] tiles, 8 indirect DMAs instead of 32);
    section bmm + MLP fused as in v1 with paired-batch (128-partition)
    local/sec tiles.
"""

import numpy as np
import ml_dtypes
from contextlib import ExitStack

import concourse.bass as bass
import concourse.tile as tile
from concourse import bacc
from concourse import mybir
from concourse.bass import ds
from concourse.bass_utils import run_bass_kernel_spmd

F32 = mybir.dt.float32
BF16 = mybir.dt.bfloat16
I32 = mybir.dt.int32
AF = mybir.ActivationFunctionType
OP = mybir.AluOpType

P = 128


class Cfg:
    def __init__(self, S=1024):
        self.S = S          # sequence length
        self.B = 8          # batch per core
        self.I = 512        # input features
        self.H = 256        # hidden per direction
        self.G = 3 * self.H # gate features (r, z, n)
        self.MLP = 512
        self.K = 64         # sections
        self.CH = 16        # scan steps per loop iteration
        self.NT = self.S * self.B
        self.NI = self.S // self.CH
        self.nI = self.I // P   # 4  input chunks
        self.nG = self.G // P   # 6  gate chunks
        self.nH = self.H // P   # 2  hidden chunks
        self.nM = self.MLP // P # 4
        self.TT = 512           # tokens per post-phase tile
        self.nTT = self.NT // self.TT


def build_program(cfg: Cfg):
    c = cfg
    nc = bacc.Bacc("TRN2", target_bir_lowering=False, debug=False)

    io = {}
    io["xT"] = nc.dram_tensor("xT", [c.I, c.NT], BF16, kind="ExternalInput").ap()
    for d in "fb":
        io[f"wihT_{d}"] = nc.dram_tensor(f"wihT_{d}", [c.I, c.G], BF16,
                                         kind="ExternalInput").ap()
        io[f"whhT_{d}"] = nc.dram_tensor(f"whhT_{d}", [c.H, c.G], BF16,
                                         kind="ExternalInput").ap()
        # bias stationaries: rz bias rows [4,128], n-recurrent bias rows [2,128]
        io[f"brz_{d}"] = nc.dram_tensor(f"brz_{d}", [4, P], BF16,
                                        kind="ExternalInput").ap()
        io[f"bn_{d}"] = nc.dram_tensor(f"bn_{d}", [2, P], BF16,
                                       kind="ExternalInput").ap()
        # xi_n bias (b_ih n-part), per h-chunk column
        io[f"bxin_{d}"] = nc.dram_tensor(f"bxin_{d}", [P, c.nH], F32,
                                         kind="ExternalInput").ap()
    # indicator moving operands for the bias matmuls
    io["ind_rz"] = nc.dram_tensor("ind_rz", [4, c.CH * 32], BF16,
                                  kind="ExternalInput").ap()
    io["ind_n"] = nc.dram_tensor("ind_n", [2, c.CH * 16], BF16,
                                 kind="ExternalInput").ap()
    io["w1T"] = nc.dram_tensor("w1T", [4 * c.H, c.MLP], BF16, kind="ExternalInput").ap()
    io["b1"] = nc.dram_tensor("b1", [P, c.nM], F32, kind="ExternalInput").ap()
    io["w2T"] = nc.dram_tensor("w2T", [P, c.nM], BF16, kind="ExternalInput").ap()
    io["secT"] = nc.dram_tensor("secT", [c.B, c.K, c.S], BF16,
                                kind="ExternalInput").ap()
    # gather row indices, grouped [dir(2), set(2), pair(4), 128]
    io["gidx"] = nc.dram_tensor("gidx", [2, 2, 4, P, 1], I32,
                                kind="ExternalInput").ap()
    io["b2v"] = nc.dram_tensor("b2v", [1, 1], F32, kind="ExternalInput").ap()
    io["out"] = nc.dram_tensor("out", [c.NT, 1], F32, kind="ExternalOutput").ap()
    for d in "fb":
        io[f"hT_{d}"] = nc.dram_tensor(f"hT_{d}", [c.H, c.NT], BF16,
                                       kind="Internal").ap()
        io[f"h_tok_{d}"] = nc.dram_tensor(f"h_tok_{d}", [c.NT + 8, c.H], BF16,
                                          kind="Internal").ap()

    with tile.TileContext(nc) as tc:
        _body(tc, c, io)
    nc.compile()
    return nc


def _body(tc, c, io):
    nc = tc.nc
    dirs = "fb"
    hT = {d: io[f"hT_{d}"] for d in dirs}
    h_tok = {d: io[f"h_tok_{d}"] for d in dirs}
    HB = c.nH * c.B   # 16 cols: one step of one dir, (ch, b)

    with ExitStack() as octx:
        # -------- persistent across phases --------
        wpool = octx.enter_context(tc.tile_pool(name="weights", bufs=1))
        wih_sb = {d: [wpool.tile([P, c.G], BF16, tag=f"wih{d}{k}", name=f"wih{d}{k}")
                      for k in range(c.nI)] for d in dirs}
        whh_sb = {d: [wpool.tile([P, c.G], BF16, tag=f"whh{d}{k}", name=f"whh{d}{k}")
                      for k in range(c.nH)] for d in dirs}
        brz_sb = {d: wpool.tile([4, P], BF16, tag=f"brz{d}", name=f"brz{d}") for d in dirs}
        bn_sb = {d: wpool.tile([2, P], BF16, tag=f"bn{d}", name=f"bn{d}") for d in dirs}
        bxin_sb = {d: wpool.tile([P, c.nH], F32, tag=f"bxin{d}", name=f"bxin{d}")
                   for d in dirs}
        ind_rz_sb = wpool.tile([4, c.CH * 32], BF16, tag="indrz", name="indrz")
        ind_n_sb = wpool.tile([2, c.CH * 16], BF16, tag="indn", name="indn")
        b2_sb = wpool.tile([1, 1], F32, tag="b2", name="b2")
        # phase-3 weights, loaded up-front so they overlap the scan
        nMI = 4 * c.H // P
        w1_sb = [wpool.tile([P, c.MLP], BF16, tag=f"w1_{i}", name=f"w1_{i}")
                 for i in range(nMI)]
        b1_sb = wpool.tile([P, c.nM], F32, tag="b1", name="b1")
        w2_sb = wpool.tile([P, c.nM], BF16, tag="w2", name="w2")

        for d in dirs:
            for k in range(c.nI):
                nc.sync.dma_start(wih_sb[d][k][:],
                                  io[f"wihT_{d}"][k * P:(k + 1) * P, :])
            for k in range(c.nH):
                nc.sync.dma_start(whh_sb[d][k][:],
                                  io[f"whhT_{d}"][k * P:(k + 1) * P, :])
            nc.sync.dma_start(brz_sb[d][:], io[f"brz_{d}"][:])
            nc.sync.dma_start(bn_sb[d][:], io[f"bn_{d}"][:])
            nc.sync.dma_start(bxin_sb[d][:], io[f"bxin_{d}"][:])
        nc.sync.dma_start(ind_rz_sb[:], io["ind_rz"][:])
        nc.sync.dma_start(ind_n_sb[:], io["ind_n"][:])
        nc.sync.dma_start(b2_sb[:], io["b2v"][:])
        for i in range(nMI):
            nc.scalar.dma_start(w1_sb[i][:], io["w1T"][i * P:(i + 1) * P, :])
        nc.scalar.dma_start(b1_sb[:], io["b1"][:])
        nc.scalar.dma_start(w2_sb[:], io["w2T"][:])

        spool = octx.enter_context(tc.tile_pool(name="state", bufs=1))
        hT_st = {d: spool.tile([P, HB], BF16, tag=f"hst{d}", name=f"hst{d}") for d in dirs}
        for d in dirs:
            nc.vector.memset(hT_st[d][:], 0.0)
        # zero pad rows of h_tok (rows NT..NT+7 read by OOB gather indices)
        zpad = spool.tile([8, c.H], BF16, tag="zpad", name="zpad")
        nc.vector.memset(zpad[:], 0.0)
        for d in dirs:
            nc.sync.dma_start(h_tok[d][c.NT:c.NT + 8, :], zpad[:])

        lpool = octx.enter_context(tc.tile_pool(name="loc", bufs=1))

        # ================= phase 1: proj + scan + flush/transpose =========
        with ExitStack() as ctx:
            xpool = ctx.enter_context(tc.tile_pool(name="xtiles", bufs=2))
            xinpool = ctx.enter_context(tc.tile_pool(name="xin", bufs=2))
            hapool = ctx.enter_context(tc.tile_pool(name="hacc", bufs=2))
            trpool = ctx.enter_context(tc.tile_pool(name="trp", bufs=2))
            gpool = ctx.enter_context(tc.tile_pool(name="gates", bufs=4))
            rz_ps = ctx.enter_context(
                tc.tile_pool(name="rzps", bufs=2, space="PSUM"))
            n_ps = ctx.enter_context(
                tc.tile_pool(name="nps", bufs=2, space="PSUM"))
            scr_ps = ctx.enter_context(
                tc.tile_pool(name="scrps", bufs=2, space="PSUM"))

            with tc.For_i(0, c.NI) as it:
                tok0 = {"f": it * P, "b": (c.NI - 1) * P - it * P}

                # ---- psum tiles for this iter ----
                rz = {d: rz_ps.tile([P, c.CH * 32], F32, tag=f"rz{d}", name=f"rz{d}")
                      for d in dirs}
                nb = n_ps.tile([P, c.CH * 16 * 2], F32, tag="nb", name="nb")
                scr = scr_ps.tile([P, 512], F32, tag="scr", name="scr")

                # ---- bias preload matmuls ----
                for d in dirs:
                    nc.tensor.matmul(rz[d][:], brz_sb[d][:], ind_rz_sb[:],
                                     start=True, stop=False,
                                     skip_group_check=True)
                nof = {"f": 0, "b": c.CH * 16}
                for di, d in enumerate(dirs):
                    nc.tensor.matmul(nb[:, nof[d]:nof[d] + c.CH * 16],
                                     bn_sb[d][:], ind_n_sb[:],
                                     start=(di == 0), stop=False,
                                     skip_group_check=True)

                # ---- x tile DMA + input projection ----
                # hacc/xi_n layout: cols = ch*128 + s*8 + b (ch-major)
                xi_n = {}
                for di, d in enumerate(dirs):
                    xk = xpool.tile([P, c.nI * P], BF16, tag=f"x{d}", name=f"x{d}")
                    src = io["xT"].rearrange("(k p) n -> p k n", k=c.nI)
                    nc.sync.dma_start(
                        xk[:].rearrange("p (k n) -> p k n", k=c.nI),
                        src[:, :, ds(tok0[d], P)])
                    # rz chunks m=0..3 accumulate straight into the rz bank,
                    # strided per-step: col = s*32 + m*8 + b
                    rz_v = rz[d][:].rearrange("p (s m x) -> p s m x",
                                              s=c.CH, m=4)
                    for m in range(4):
                        for k in range(c.nI):
                            nc.tensor.matmul(
                                rz_v[:, :, m, :],
                                wih_sb[d][k][:, m * P:(m + 1) * P],
                                xk[:, k * P:(k + 1) * P],
                                start=False, stop=False,
                                skip_group_check=True)
                    # n chunks m=4,5 -> scratch psum then SBUF with bias
                    so = di * 256
                    for m in (4, 5):
                        for k in range(c.nI):
                            nc.tensor.matmul(
                                scr[:, so + (m - 4) * P:so + (m - 3) * P],
                                wih_sb[d][k][:, m * P:(m + 1) * P],
                                xk[:, k * P:(k + 1) * P],
                                start=(di == 0 and m == 4 and k == 0),
                                stop=(di == 1 and m == 5 and k == c.nI - 1),
                                skip_group_check=True)
                    xin = xinpool.tile([P, c.CH * HB], BF16, tag=f"xin{d}",
                                       name=f"xin{d}")
                    for ch in range(c.nH):
                        nc.scalar.activation(xin[:, ch * P:(ch + 1) * P],
                                             scr[:, so + ch * P:so + (ch + 1) * P],
                                             AF.Identity,
                                             bias=bxin_sb[d][:, ch:ch + 1])
                    xi_n[d] = xin

                # ---- scan: CH steps, fwd & bwd interleaved ----
                hacc = {d: hapool.tile([P, c.CH * HB], BF16,
                                       tag=f"ha{d}", name=f"ha{d}") for d in dirs}
                hacc_v = {d: hacc[d][:].rearrange("p (c s x) -> p c s x",
                                                  c=c.nH, s=c.CH) for d in dirs}
                for s in range(c.CH):
                    for d in dirs:
                        sd = s if d == "f" else c.CH - 1 - s
                        if s == 0:
                            prev = hT_st[d][:].rearrange("p (c x) -> p c x",
                                                         c=c.nH)
                        else:
                            pd = sd - 1 if d == "f" else sd + 1
                            prev = hacc_v[d][:, :, pd, :]
                        # 12 recurrent matmuls accumulate onto bias+proj
                        last_iter_mm = (s == c.CH - 1)
                        for m in range(4):
                            for ci in range(c.nH):
                                nc.tensor.matmul(
                                    rz[d][:, sd * 32 + m * 8:sd * 32 + m * 8 + 8],
                                    whh_sb[d][ci][:, m * P:(m + 1) * P],
                                    prev[:, ci, :],
                                    start=False,
                                    stop=(last_iter_mm and m == 3 and ci == 1),
                                    skip_group_check=True)
                        for m in (4, 5):
                            for ci in range(c.nH):
                                nc.tensor.matmul(
                                    nb[:, nof[d] + sd * 16 + (m - 4) * 8:
                                        nof[d] + sd * 16 + (m - 3) * 8],
                                    whh_sb[d][ci][:, m * P:(m + 1) * P],
                                    prev[:, ci, :],
                                    start=False,
                                    stop=(d == "b" and last_iter_mm
                                          and m == 5 and ci == 1),
                                    skip_group_check=True)
                        # elementwise chain (7 ops)
                        sig = gpool.tile([P, 32], BF16, tag=f"sig{d}", name=f"sig{d}")
                        nc.scalar.activation(sig[:], rz[d][:, sd * 32:sd * 32 + 32],
                                             AF.Sigmoid)
                        t1 = gpool.tile([P, HB], BF16, tag=f"t1{d}", name=f"t1{d}")
                        nc.vector.tensor_tensor(
                            t1[:], nb[:, nof[d] + sd * 16:nof[d] + sd * 16 + 16],
                            sig[:, 0:16], OP.mult)
                        t2 = gpool.tile([P, HB], BF16, tag=f"t2{d}", name=f"t2{d}")
                        nc.vector.tensor_tensor(
                            t2[:].rearrange("p (c x) -> p c x", c=c.nH),
                            t1[:].rearrange("p (c x) -> p c x", c=c.nH),
                            xi_n[d][:].rearrange("p (c s x) -> p c s x",
                                                 c=c.nH, s=c.CH)[:, :, sd, :],
                            OP.add)
                        n_t = gpool.tile([P, HB], BF16, tag=f"n{d}", name=f"n{d}")
                        nc.scalar.activation(n_t[:], t2[:], AF.Tanh)
                        n_v = n_t[:].rearrange("p (c x) -> p c x", c=c.nH)
                        dt_ = gpool.tile([P, HB], BF16, tag=f"dt{d}", name=f"dt{d}")
                        dt_v = dt_[:].rearrange("p (c x) -> p c x", c=c.nH)
                        nc.vector.tensor_tensor(dt_v, prev, n_v, OP.subtract)
                        et = gpool.tile([P, HB], BF16, tag=f"et{d}", name=f"et{d}")
                        nc.vector.tensor_tensor(et[:], sig[:, 16:32], dt_[:], OP.mult)
                        nc.vector.tensor_tensor(
                            hacc_v[d][:, :, sd, :], n_v,
                            et[:].rearrange("p (c x) -> p c x", c=c.nH), OP.add)

                # ---- end of iter: state, flush, transpose ----
                for d in dirs:
                    last = c.CH - 1 if d == "f" else 0
                    nc.scalar.activation(
                        hT_st[d][:].rearrange("p (c x) -> p c x", c=c.nH),
                        hacc_v[d][:, :, last, :], AF.Identity)
                    # flush to hT[d]: dst rows (ch*128+p), cols tok0 + s*8+b
                    dst = hT[d].rearrange("(ch p) n -> p ch n",
                                          ch=c.nH)[:, :, ds(tok0[d], P)]
                    src = hacc[d][:].rearrange("p (ch sx) -> p ch sx", ch=c.nH)
                    nc.sync.dma_start(dst, src)
                    # token-major transpose for the gathers: SBUF->SBUF
                    # xbar transpose of each contiguous [128,128] ch-slice
                    for ch in range(c.nH):
                        tr = trpool.tile([P, P], BF16, tag=f"tr{d}{ch}",
                                         name=f"tr{d}{ch}")
                        nc.scalar.dma_start_transpose(
                            tr[:], hacc[d][:, ch * P:(ch + 1) * P])
                        nc.scalar.dma_start(
                            h_tok[d][ds(tok0[d], P), ch * P:(ch + 1) * P],
                            tr[:])

        tc.strict_bb_all_engine_barrier()

        # ================= phase 2: gathers + local features =============
        # local2[j] [128, 512]: rows = (b0+{0,1} batch pair) x 64 sections,
        # cols = [fe-fb | bb-be] halves
        local2 = [lpool.tile([P, 2 * c.H], BF16, tag=f"loc{j}", name=f"loc{j}")
                  for j in range(4)]
        loc_hi = [lpool.tile([c.K, 2 * c.H], BF16, tag=f"lhi{j}", name=f"lhi{j}")
                  for j in range(4)]
        with ExitStack() as ctx:
            gxpool = ctx.enter_context(tc.tile_pool(name="gx", bufs=1))
            gt = {}
            for di, d in enumerate(dirs):
                for st in range(2):  # set 0: end/begin-first, set 1: other
                    for j in range(4):
                        idx = gxpool.tile([P, 1], I32, tag=f"gi{d}{st}{j}",
                                          name=f"gi{d}{st}{j}")
                        nc.sync.dma_start(idx[:], io["gidx"][di, st, j, :, :])
                        g = gxpool.tile([P, c.H], BF16, tag=f"g{d}{st}{j}",
                                        name=f"g{d}{st}{j}")
                        nc.gpsimd.indirect_dma_start(
                            out=g[:], out_offset=None, in_=h_tok[d][:],
                            in_offset=bass.IndirectOffsetOnAxis(ap=idx[:, :1],
                                                                axis=0),
                            bounds_check=c.NT + 7, oob_is_err=False)
                        gt[(d, st, j)] = g
            for j in range(4):
                # fwd half: g_end - g_begin ; bwd half: g_begin - g_end
                nc.vector.tensor_tensor(local2[j][:, 0:c.H],
                                        gt[("f", 0, j)][:], gt[("f", 1, j)][:],
                                        OP.subtract)
                nc.vector.tensor_tensor(local2[j][:, c.H:],
                                        gt[("b", 0, j)][:], gt[("b", 1, j)][:],
                                        OP.subtract)
            # odd-batch halves live at partitions 64:128, but matmul
            # stationaries must start at partition 0: shift them down
            for j in range(4):
                nc.sync.dma_start(loc_hi[j][:], local2[j][c.K:, :])

        tc.strict_bb_all_engine_barrier()

        # ================= phase 3: fused bmm + MLP =================
        with ExitStack() as ctx:
            mpool = ctx.enter_context(tc.tile_pool(name="mlp", bufs=2))
            l_psum = ctx.enter_context(tc.tile_pool(name="lps", bufs=2, space="PSUM"))
            h1_psum = ctx.enter_context(tc.tile_pool(name="h1ps", bufs=2, space="PSUM"))
            o_psum = ctx.enter_context(tc.tile_pool(name="ops", bufs=2, space="PSUM"))
            secpool = ctx.enter_context(tc.tile_pool(name="sec", bufs=2))

            TB = c.TT // c.B      # 64 tokens-per-batch per tile
            nLC = 2 * c.H // P    # 4 lcr chunks
            for j in range(c.nTT):
                sec_sb = [secpool.tile([c.K, TB], BF16, tag=f"sec{b}", name=f"sec{b}")
                          for b in range(c.B)]
                for b in range(c.B):
                    nc.sync.dma_start(sec_sb[b][:],
                                      io["secT"][b, :, j * TB:(j + 1) * TB])
                lcr = [mpool.tile([P, c.TT], BF16, tag=f"lcr{fc}", name=f"lcr{fc}")
                       for fc in range(nLC)]
                for fc in range(nLC):
                    ps = l_psum.tile([P, c.TT], F32, tag="lps", name="lps")
                    for b in range(c.B):
                        pj, sub = b // 2, b % 2
                        loc = (local2[pj][0:c.K, fc * P:(fc + 1) * P]
                               if sub == 0 else
                               loc_hi[pj][:, fc * P:(fc + 1) * P])
                        nc.tensor.matmul(
                            ps[:, b * TB:(b + 1) * TB],
                            loc, sec_sb[b][:],
                            start=True, stop=True)
                    src = ps[:].rearrange("p (b u) -> p b u", b=c.B)
                    dstv = lcr[fc][:].rearrange("p (u b) -> p b u", b=c.B)
                    if fc % 2 == 0:
                        nc.scalar.activation(dstv, src, AF.Copy)
                    else:
                        nc.vector.tensor_copy(dstv, src)
                rhs = []
                for d in dirs:
                    for chn in range(c.nH):
                        t = mpool.tile([P, c.TT], BF16, tag=f"hin{d}{chn}",
                                       name=f"hin{d}{chn}")
                        nc.sync.dma_start(
                            t[:], hT[d][chn * P:(chn + 1) * P,
                                        j * c.TT:(j + 1) * c.TT])
                        rhs.append(t)
                rhs.extend(lcr)
                h1 = []
                for mc in range(c.nM):
                    ps = h1_psum.tile([P, c.TT], F32, tag="h1ps", name="h1ps")
                    for icx in range(nMI):
                        nc.tensor.matmul(ps[:], w1_sb[icx][:, mc * P:(mc + 1) * P],
                                         rhs[icx][:], start=(icx == 0),
                                         stop=(icx == nMI - 1))
                    h1t = mpool.tile([P, c.TT], BF16, tag=f"h1_{mc}", name=f"h1_{mc}")
                    nc.scalar.activation(h1t[:], ps[:], AF.Relu,
                                         bias=b1_sb[:, mc:mc + 1])
                    h1.append(h1t)
                pso = o_psum.tile([1, c.TT], F32, tag="ops", name="ops")
                for mc in range(c.nM):
                    nc.tensor.matmul(pso[:], w2_sb[:, mc:mc + 1], h1[mc][:],
                                     start=(mc == 0), stop=(mc == c.nM - 1))
                ot = mpool.tile([1, c.TT], F32, tag="ot", name="ot")
                nc.scalar.activation(ot[:], pso[:], AF.Identity,
                                     bias=b2_sb[0:1, 0:1])
                nc.sync.dma_start(io["out"][j * c.TT:(j + 1) * c.TT, :], ot[:])


# ======================= host side =======================

def _prep_core(inputs_np, core, c):
    bf = ml_dtypes.bfloat16
    bsl = slice(core * c.B, (core + 1) * c.B)
    x = inputs_np["inputs"][:, bsl, :]
    feed = {}
    feed["xT"] = np.ascontiguousarray(
        x.transpose(2, 0, 1).reshape(c.I, c.NT)).astype(bf)
    for d, sfx in (("f", "_f"), ("b", "_b")):
        wih = inputs_np["W_ih" + sfx]
        whh = inputs_np["W_hh" + sfx]
        bih = inputs_np["b_ih" + sfx].astype(np.float32)
        bhh = inputs_np["b_hh" + sfx].astype(np.float32)
        feed[f"wihT_{d}"] = np.ascontiguousarray(wih.T).astype(bf)
        feed[f"whhT_{d}"] = np.ascontiguousarray(whh.T).astype(bf)
        brz = (bih + bhh)[:2 * c.H]
        feed[f"brz_{d}"] = np.ascontiguousarray(brz.reshape(4, P)).astype(bf)
        feed[f"bn_{d}"] = np.ascontiguousarray(
            bhh[2 * c.H:].reshape(2, P)).astype(bf)
        feed[f"bxin_{d}"] = np.ascontiguousarray(
            bih[2 * c.H:].reshape(c.nH, P).T)
    # indicator matmul moving operands
    col = np.arange(c.CH * 32)
    feed["ind_rz"] = ((col % 32) // 8 == np.arange(4)[:, None]).astype(bf)
    col = np.arange(c.CH * 16)
    feed["ind_n"] = ((col % 16) // 8 == np.arange(2)[:, None]).astype(bf)

    feed["w1T"] = np.ascontiguousarray(inputs_np["W1"].T).astype(bf)
    feed["b1"] = np.ascontiguousarray(
        inputs_np["b1"].astype(np.float32).reshape(c.nM, P).T)
    feed["w2T"] = np.ascontiguousarray(
        inputs_np["W2"].reshape(c.MLP).reshape(c.nM, P).T).astype(bf)
    feed["b2v"] = np.array([[float(np.asarray(inputs_np["b2"]).reshape(-1)[0])]],
                           np.float32)
    feed["secT"] = np.ascontiguousarray(
        inputs_np["section_indicator"][bsl].transpose(0, 2, 1)).astype(bf)
    beg = np.asarray(inputs_np["begin"][bsl]).astype(np.int64)
    end = np.asarray(inputs_np["end"][bsl]).astype(np.int64)
    BIG = c.NT
    bvec = np.arange(c.B)[:, None]

    def rows(v):
        return np.where(v > 0, (v - 1) * c.B + bvec, BIG).astype(np.int32)

    # [dir, set, b, k]: fwd uses (end, begin), bwd uses (begin, end)
    gi = np.stack([np.stack([rows(end), rows(beg)]),
                   np.stack([rows(beg), rows(end)])])
    feed["gidx"] = np.ascontiguousarray(gi.reshape(2, 2, 4, P, 1))
    return feed


_PROG_CACHE = {}
LAST_RESULTS = None


def _get_prog(c: Cfg):
    if c.S not in _PROG_CACHE:
        _PROG_CACHE[c.S] = build_program(c)
    return _PROG_CACHE[c.S]


def kernel(**inputs):
    c = Cfg(S=np.asarray(inputs["inputs"]).shape[0])
    inputs_np = {k: np.asarray(v) for k, v in inputs.items()}
    global LAST_RESULTS
    nc = _get_prog(c)
    in_maps = [_prep_core(inputs_np, core, c) for core in range(8)]
    res = run_bass_kernel_spmd(nc, in_maps, core_ids=list(range(8)))
    LAST_RESULTS = res
    outs = [res.results[core]["out"].reshape(c.S, c.B, 1) for core in range(8)]
    return np.concatenate(outs, axis=1).astype(np.float32)
